# revision 1
# baseline (speedup 1.0000x reference)
"""Trainium2 Bass kernel for nn_MultiHeadAttention_8684423872640.

Math: the reference collapses algebraically. With
  s[m]   = Wfc[0, m // 64] / sqrt(64)
  Abar   = (Wk * s[:,None]).T @ Wq / L          # [1024, 1024] weights-only
  u      = Wk.T @ (s * bq)                      # [1024]
  qv     = Wq.T @ (s * bk) / L                  # [1024]
  c0     = (s * bk) @ bq + bfc[0]
the output for batch b is
  xsum_b = sum_l x[b, l, :]                     # [1024]
  w_eff  = Abar @ xsum_b + u                    # [1024]
  c      = qv @ xsum_b + c0
  out[b, l, 0] = x[b, l, :] @ w_eff + c

Sharding: data-parallel over B — core c handles batch c.

Final pipeline (per core), ~45.6us vs the 66us baseline:
  - x ships as fp8-e4m3 (4 MiB; e3m4's denormal range covers 20% of
    N(0,1) and measured 8e-2 rel-err on HW). Abar/qv ship fp8-e4m3
    pre-scaled x2^20 (entries ~1e-7 underflow unscaled); u f32 scaled.
  - DMA: <=8 large transfers (tile pairs via 3-level APs) so every
    consumer waits a first-use semaphore threshold; issues pinned to
    queue fronts with tc.high_priority; SWDGE carries only qv/u/c0.
  - Row-sums at measured fp8 engine rates: ACT activation-accum eats
    [0:2176] raw per tile, DVE tensor_reduce (raw fp8 — HW-verified
    exact) takes the rest, one 2D-output reduce per pair-DMA; GpSimd
    does the fp32+fp32->bf16 combine.
  - Folds: closed per-MM PSUM groups + DVE accumulation (open
    interleaved start-groups silently drop all but the last write);
    3 filler MMs per early fold + two warmup bursts hold the PE HAM
    clock at 2.4 GHz through to pass-2.
  - c broadcast via PE ones-matmul + ACT copy (gpsimd's
    partition_broadcast triggers a Q7 library swap, ~6us late).
  - Pass-2: 4-way column-tiled matvec (tile_position=(0,32j)), mixed
    dtype (bf16 w_eff stationary x fp8 moving) at the warm 379ns/MM
    floor; full-partition epilogues; one merged out-DMA.
"""

import os
import sys
import functools
import numpy as np

B, L, N = 8, 4096, 1024
D_K = 64
NCORES = 8
PT = N // 128   # 8 feature tiles
LCH = 512       # pass-2 moving chunk (PSUM bank limit)
QW = L // 4     # tail-tile DMA quarter

# row-sum slice widths per full tile (DVE / ACT / GPS)



_TRN_REPO = "/opt/trn_rl_repo"


def _ensure_path():
    if _TRN_REPO not in sys.path and os.path.isdir(_TRN_REPO):
        sys.path.insert(0, _TRN_REPO)


# pass-2 w_eff dtype: 'mixed' = bf16 stationary (x stays fp8 moving);
# 'fp8' = w cast to e4m3 scaled x128 (both operands fp8)
_W_MODE = os.environ.get("KERNEL_W_MODE", "mixed")


@functools.lru_cache(maxsize=2)
def _build(w_mode: str = _W_MODE, warm1: int = 8, warm2: int = 6):
    _ensure_path()
    import concourse.bass as bass
    import concourse.tile as tile
    from concourse import bacc, mybir

    f32 = mybir.dt.float32
    bf16 = mybir.dt.bfloat16
    f8 = mybir.dt.float8e4
    wdt = bf16
    # Abar/qv/u pre-scaled x2^20 on host so Abar fits fp8e4 (its entries
    # ~1e-7 underflow unscaled); the epilogue divides back out.
    wscale = float(2 ** 20)

    nc = bacc.Bacc(
        "TRN2",
        target_bir_lowering=False,
        debug=False,
        enable_asserts=False,
        num_devices=NCORES,
    )

    xT = nc.dram_tensor("xT", [N, L], f8, kind="ExternalInput").ap()
    atr = nc.dram_tensor("atr", [128, PT * N], f8, kind="ExternalInput").ap()
    qv8 = nc.dram_tensor("qv8", [128, PT], f8, kind="ExternalInput").ap()
    u8 = nc.dram_tensor("u8", [128, PT], f32, kind="ExternalInput").ap()
    c0 = nc.dram_tensor("c0", [1, 1], f32, kind="ExternalInput").ap()
    out_d = nc.dram_tensor("out", [1, L], f32, kind="ExternalOutput").ap()

    with tile.TileContext(nc) as tc:
        with (
            tc.tile_pool(name="xpool", bufs=PT) as xpool,
            tc.tile_pool(name="cpool", bufs=1) as cpool,
            tc.tile_pool(name="spool", bufs=4) as spool,
            tc.tile_pool(name="xsums", bufs=PT + 6) as xsums,
            tc.tile_pool(name="scrp", bufs=3) as scr_p,
            tc.tile_pool(name="gscrp", bufs=3) as gscr_p,
            tc.tile_pool(name="wps", bufs=2, space="PSUM") as wps,
            tc.tile_pool(name="cps", bufs=1, space="PSUM") as cps,
            tc.tile_pool(name="ops", bufs=2, space="PSUM") as ops,
            tc.tile_pool(name="wrm", bufs=1, space="PSUM") as wrm,
            tc.tile_pool(name="cbp", bufs=1, space="PSUM") as cbp,
        ):
            x_pr = [xpool.tile([128, 2 * L], f8, tag="x", name=f"xp{k}")
                    for k in range(3)]
            x6_t = xpool.tile([128, L], f8, tag="x", name="x6")
            x7_t = xpool.tile([128, L], f8, tag="x", name="x7")
            x_sb = [x_pr[i // 2][:, (i % 2) * L:(i % 2 + 1) * L]
                    for i in range(6)] + [x6_t[:], x7_t[:]]
            at_sb = cpool.tile([128, PT * N], f8, tag="at")
            qv_sb = cpool.tile([128, PT], f8, tag="qv")
            u_sb = cpool.tile([128, PT], f32, tag="u")
            c0_sb = cpool.tile([1, 1], f32, tag="c0")
            ones32 = cpool.tile([1, 128], f32, tag="ones")
            nc.gpsimd.memset(ones32[:], 1.0)

            # ---- DMA: few, large transfers; issues pinned to queue front.
            # Tile pairs ride one dma_start each (3-level AP) so every
            # consumer's semaphore wait is a first-use threshold.
            half = PT * N // 2
            H = L // 2
            def xpair(eng, k):
                eng.dma_start(
                    x_pr[k].rearrange("p (s l) -> p s l", s=2),
                    xT[256 * k:256 * (k + 1), :]
                    .rearrange("(s p) l -> p s l", s=2))

            with tc.high_priority():
                nc.gpsimd.dma_start(qv_sb[:], qv8[:])
                nc.gpsimd.dma_start(u_sb[:], u8[:])
                nc.gpsimd.dma_start(c0_sb[:], c0[:])
                # sync ring: x0, x1, at (1 MiB fp8), x6 (2.5 MiB)
                # x0/x1 as singles so the reduce pipeline starts earlier
                nc.sync.dma_start(x_pr[0][:, 0:L], xT[0:128, :])
                nc.sync.dma_start(x_pr[0][:, L:], xT[128:256, :])
                nc.sync.dma_start(at_sb[:], atr[:])
                nc.sync.dma_start(x6_t[:], xT[768:896, :])
                # scalar ring: x23, x45, x7a, x7b       (2.5 MiB)
                xpair(nc.scalar, 1)
                xpair(nc.scalar, 2)
                nc.scalar.dma_start(x7_t[:, 0:H], xT[896:, 0:H])
                nc.scalar.dma_start(x7_t[:, H:], xT[896:, H:])

            # ---- row-sum helpers ----
            # Engine rates (HW-measured, fp8 in): ACT activation-accum
            # 0.76 ns/elem; DVE tensor_reduce ~0.85 ns/elem. Each tile:
            # ACT eats [0:AW] raw, DVE raw-reduces the rest, GPS does the
            # final combine+cast (fp32+fp32 -> bf16 xm in one op).
            AW = 2176           # ACT raw share of a full tile

            # NOTE (HW-verified): per-column start=True MMs with the group
            # left open across interleaved columns lose all but the last
            # start-write. Use closed per-MM groups + DVE accumulation.
            c_ps = cps.tile([1, 1], f32, tag="cps")
            w8_acc = spool.tile([128, PT], f32, tag="w8acc")

            def fold(pt, xm):
                wp = wps.tile([128, PT], f32, tag="wp", name=f"wp{pt}")
                for nt in range(PT):
                    nc.tensor.matmul(
                        wp[:, nt:nt + 1],
                        at_sb[:, pt * N + nt * 128: pt * N + (nt + 1) * 128],
                        xm, start=True, stop=True)
                nc.tensor.matmul(
                    c_ps[:], qv_sb[:, pt:pt + 1], xm,
                    start=(pt == 0), stop=(pt == PT - 1))
                if pt == 0:
                    nc.vector.tensor_copy(w8_acc[:], wp[:])
                else:
                    nc.vector.tensor_add(w8_acc[:], w8_acc[:], wp[:])
                # PE filler gated on the same xm: keeps the HAM clock warm
                # through the reduce window without blocking later folds
                if pt < 6:
                    for i in range(3):
                        nc.tensor.matmul(
                            wscr[:], xm, x_sb[pt][:, i * LCH:(i + 1) * LCH],
                            start=(i == 0), stop=(i == 2))

            def finish_tile(pt, parts):
                """xm = bf16(parts0 + parts1) on GPS, then fold."""
                xm = xsums.tile([128, 1], bf16, tag="xm", name=f"xm{pt}")
                nc.gpsimd.tensor_add(xm[:], parts[:, 0:1], parts[:, 1:2])
                fold(pt, xm[:])
                return xm

            def reduce_full(pt):
                """Full tile: ACT raw [0:AW] -> p0; DVE raw tensor_reduce
                [AW:L] -> p1; GPS combines."""
                x_ = x_sb[pt]
                parts = xsums.tile([128, 2], f32, tag="pp", name=f"pp{pt}")
                nc.scalar.activation(
                    act_scr[:, 0:AW], x_[:, 0:AW],
                    mybir.ActivationFunctionType.Copy,
                    bias=0.0, accum_out=parts[:, 0:1])
                nc.vector.tensor_reduce(
                    parts[:, 1:2], x_[:, AW:L],
                    axis=mybir.AxisListType.X, op=mybir.AluOpType.add)
                return finish_tile(pt, parts)

            # ---- warmup burst 1: fire HAM as soon as x0 lands ----
            wscr = wrm.tile([1, LCH], f32, tag="warm")
            for i in range(warm1):
                nc.tensor.matmul(
                    wscr[:], x_sb[0][:, 0:1], x_sb[0][:, 0:LCH],
                    start=(i == 0), stop=(i == warm1 - 1))

            # ---- pass 1: row-sums + folds in arrival order ----
            act_scr = cpool.tile([128, AW], f8, tag="ascr")
            def reduce_pair(k):
                """Tiles 2k,2k+1 (one pair DMA): 2 ACT raw accums + ONE
                DVE 2D-output reduce over the pair's [AW:L] slices."""
                parts = xsums.tile([128, 4], f32, tag="pp", name=f"ppp{k}")
                for j in range(2):
                    nc.scalar.activation(
                        act_scr[:, 0:AW], x_sb[2 * k + j][:, 0:AW],
                        mybir.ActivationFunctionType.Copy,
                        bias=0.0, accum_out=parts[:, j:j + 1])
                xv = x_pr[k].rearrange("p (s l) -> p s l", s=2)[:, :, AW:L]
                nc.vector.tensor_reduce(
                    parts[:, 2:4], xv,
                    axis=mybir.AxisListType.X, op=mybir.AluOpType.add)
                for j in range(2):
                    pt = 2 * k + j
                    xm = xsums.tile([128, 1], bf16, tag="xm", name=f"xm{pt}")
                    nc.gpsimd.tensor_add(
                        xm[:], parts[:, j:j + 1], parts[:, 2 + j:3 + j])
                    fold(pt, xm[:])

            reduce_full(0)
            reduce_full(1)
            reduce_pair(1)   # tiles 2, 3
            reduce_pair(2)   # tiles 4, 5
            # warmup burst 2 ahead of the fold/pass-2 tail
            for i in range(warm2):
                nc.tensor.matmul(
                    wscr[:], x_sb[6][:, H:H + 1], x_sb[6][:, H:H + LCH],
                    start=(i == 0), stop=(i == warm2 - 1))
            xm6 = reduce_full(6)

            # tile 7, two halves: ACT raw-accumulates 7a; DVE chain 7b
            parts7 = xsums.tile([128, 2], f32, tag="pp", name="pp7")
            nc.scalar.activation(
                act_scr[:, 0:H], x_sb[7][:, 0:H],
                mybir.ActivationFunctionType.Copy,
                bias=0.0, accum_out=parts7[:, 0:1])
            nc.vector.tensor_reduce(
                parts7[:, 1:2], x_sb[7][:, H:L],
                axis=mybir.AxisListType.X, op=mybir.AluOpType.add)
            finish_tile(7, parts7)

            # ---- finalize w_eff / c ----
            w_sb = spool.tile([128, PT], wdt, tag="weff")
            nc.vector.tensor_add(w_sb[:], w8_acc[:], u_sb[:])
            c_sb = spool.tile([1, 1], f32, tag="csb")
            nc.vector.tensor_scalar(
                c_sb[:], c_ps[:], 1.0 / wscale, c0_sb[0:1, 0:1],
                mybir.AluOpType.mult, mybir.AluOpType.add)
            # broadcast c to all partitions via PE (ones.T @ c) + ACT
            # copy to SBUF -- gpsimd.partition_broadcast needs a Q7 library
            # swap and measured ~6us late, gating the epilogues
            cb_ps = cbp.tile([128, 1], f32, tag="cbps")
            nc.tensor.matmul(cb_ps[:], ones32[:], c_sb[:], start=True, stop=True)
            c_bc = spool.tile([128, 1], f32, tag="cbc")
            nc.scalar.activation(
                c_bc[:], cb_ps[:], mybir.ActivationFunctionType.Copy, bias=0.0)

            # ---- pass 2: 4-way column-tiled matvec, 2 waves ----
            out_sb = cpool.tile([128, 2 * LCH], f32, tag="osb")
            for wave in range(2):
                o_ps = ops.tile([128, LCH], f32, tag="ops", name=f"o{wave}")
                for nt in range(PT):
                    for j in range(4):
                        lc = wave * 4 + j
                        nc.tensor.matmul(
                            o_ps[32 * j:32 * j + 1, :],
                            w_sb[:, nt:nt + 1],
                            x_sb[nt][:, lc * LCH:(lc + 1) * LCH],
                            start=(nt == 0), stop=(nt == PT - 1),
                            tile_position=(0, 32 * j))
                nc.vector.tensor_scalar(
                    out_sb[:, wave * LCH:(wave + 1) * LCH],
                    o_ps[:, :],
                    1.0 / wscale, c_bc[:, 0:1],
                    mybir.AluOpType.mult, mybir.AluOpType.add)
            # one store for all 8 chunks: dram chunk c=wave*4+j from
            # sbuf row 32j, cols wave*512+k
            dst = out_d[0:1, :].rearrange("p (w j k) -> p j w k", w=2, j=4)
            src = out_sb[0:97:32, 0:2 * LCH].rearrange(
                "p (w k) -> p w k", w=2)
            nc.sync.dma_start(dst, src)

    nc.compile()
    return nc


def _prep_host(inputs, w_mode=_W_MODE):
    """Fold weights on host (f64 accumulate) and lay out per-core arrays."""
    import ml_dtypes
    wscale = float(2 ** 20)

    Wq = np.asarray(inputs["Wq"], np.float64)
    bq = np.asarray(inputs["bq"], np.float64)
    Wk = np.asarray(inputs["Wk"], np.float64)
    bk = np.asarray(inputs["bk"], np.float64)
    Wfc = np.asarray(inputs["Wfc"], np.float64)
    bfc = np.asarray(inputs["bfc"], np.float64)

    s = np.repeat(Wfc[0], D_K) / np.sqrt(D_K)
    A = (Wk * s[:, None]).T @ Wq / L          # [n, p] ; w_eff = A @ xsum + u
    u = Wk.T @ (s * bq)
    qv = Wq.T @ (s * bk) / L
    c0 = float((s * bk) @ bq + bfc[0])

    bf16 = ml_dtypes.bfloat16
    f8 = ml_dtypes.float8_e4m3

    at = np.ascontiguousarray(A.T) * wscale
    atr = np.ascontiguousarray(
        at.reshape(PT, 128, N).transpose(1, 0, 2).reshape(128, PT * N)
    ).astype(f8)
    qv8 = np.ascontiguousarray(
        (qv * wscale).reshape(PT, 128).T).astype(f8)
    u8 = np.ascontiguousarray((u * wscale).reshape(PT, 128).T).astype(np.float32)
    c0a = np.full((1, 1), c0, np.float32)

    x = np.asarray(inputs["x"])
    shared = {"atr": atr, "qv8": qv8, "u8": u8, "c0": c0a}
    in_maps = []
    for c in range(NCORES):
        m = dict(shared)
        m["xT"] = np.ascontiguousarray(x[c].T).astype(f8)
        in_maps.append(m)
    return in_maps


LAST_RESULTS = None


def kernel(**inputs) -> np.ndarray:
    global LAST_RESULTS
    _ensure_path()
    from concourse.bass_utils import run_bass_kernel_spmd

    nc = _build(_W_MODE)
    in_maps = _prep_host(inputs, _W_MODE)
    kw = {}
    if os.environ.get("KERNEL_TRACE"):
        kw["trace"] = True
    res = run_bass_kernel_spmd(nc, in_maps, list(range(NCORES)), **kw)
    LAST_RESULTS = res
    out = np.stack([res.results[c]["out"].reshape(L, 1) for c in range(NCORES)])
    return out.astype(np.float32)


if __name__ == "__main__":
    rng = np.random.default_rng(0)
    demo = {
        "x": rng.standard_normal((B, L, N), np.float32),
        "Wq": rng.standard_normal((N, N), np.float32) * 0.03,
        "bq": rng.standard_normal((N,), np.float32) * 0.03,
        "Wk": rng.standard_normal((N, N), np.float32) * 0.03,
        "bk": rng.standard_normal((N,), np.float32) * 0.03,
        "Wfc": rng.standard_normal((1, 16), np.float32) * 0.25,
        "bfc": rng.standard_normal((1,), np.float32) * 0.25,
    }
    o = kernel(**demo)
    print("out", o.shape, o.dtype, float(np.abs(o).max()))



# revision 12
# speedup vs baseline: 1.0614x; 1.0614x over previous
"""Trainium2 Bass kernel for nn_MultiHeadAttention_8684423872640.

Math: the reference collapses algebraically. With
  s[m]   = Wfc[0, m // 64] / sqrt(64)
  Abar   = (Wk * s[:,None]).T @ Wq / L          # [1024, 1024] weights-only
  u      = Wk.T @ (s * bq)                      # [1024]
  qv     = Wq.T @ (s * bk) / L                  # [1024]
  c0     = (s * bk) @ bq + bfc[0]
the output for batch b is
  xsum_b = sum_l x[b, l, :]                     # [1024]
  w_eff  = Abar @ xsum_b + u                    # [1024]
  c      = qv @ xsum_b + c0
  out[b, l, 0] = x[b, l, :] @ w_eff + c

Sharding: data-parallel over B -- core c handles batch c.

v2 pipeline (per core):
  - x ships fp8-e4m3 [N=1024, L=4096] as 4 pair tiles [128, 2L]; rings:
    sync x0,x2,x4,x6,x7b / scalar x1,x3,x5,x7a / gps at0,qv,u,c0,at1.
    All DMA issues pinned to queue fronts (descgen done before data
    arrives, so compute engines' queues are clean in steady state).
  - Row sums: per tile, three zones reduced in parallel, one op each:
    ACT activation(Copy, accum_out) on raw fp8; DVE and GPS each do
    scalar_tensor_tensor(halves, op=add, accum_out) -- pairwise add +
    free-axis accumulate in a single pass (2 cols/cycle consumed).
    GPS combines the three fp32 partials into fp16 xm in one stt op.
  - Folds: per tile pt, 8 closed-group MMs (at fp8 x xm fp16) into a
    single PSUM bank (cols pt*8+nt) + qv MM chained into cps.  One DVE
    tensor_reduce at the end sums all 64 fold columns -> w8acc, then
    one stt adds u and rescales into the pass-2 w dtype.
  - PE warmup MMs gated on x0/x2 keep the HAM clock ramping through
    the reduce window (fillers per fold as in v1).
  - c broadcast via PE ones-matmul + ACT copy.
  - Pass-2 (KERNEL_P2=dr, default): DoubleRow fp8xfp8 matvec -- 32 MMs
    of [128,2,512] pairs at tile_position (0,32j), 2 PSUM waves; w in
    fp8 scaled 2^18.  KERNEL_P2=mx falls back to v1's 64 mixed-dtype
    MMs (bf16 w, scale 2^20).
  - Epilogue per wave (tensor_scalar mult+add c) overlaps wave 2 MMs;
    two simple 4-descriptor out-DMAs on the sync ring.
"""

import os
import sys
import functools
import numpy as np

B, L, N = 8, 4096, 1024
D_K = 64
NCORES = 8
PT = N // 128   # 8 feature tiles
LCH = 512       # pass-2 moving chunk (PSUM bank limit)
H = L // 2

_TRN_REPO = "/opt/trn_rl_repo"


def _ensure_path():
    if _TRN_REPO not in sys.path and os.path.isdir(_TRN_REPO):
        sys.path.insert(0, _TRN_REPO)


# pass-2 mode: 'mx' = bf16 w stationary x fp8 moving, 4-way column-tiled
#   (4 fp8 moving cols/cycle -- the PE moving bus limit; optimal).
# 'dr' = DoubleRow fp8 x fp8: REJECTED by walrus for tile_position j>0
#   (s3d3_mm_valid_dst_partition) -- untiled DR is only 2 cols/cycle, so
#   it cannot beat 4-way mx; kept for reference.
_P2 = os.environ.get("KERNEL_P2", "mx")
# reduce zone widths per full tile (ACT raw / DVE stt / GPS stt)
# reduce zones per tile: ACT raw-accum [0:ZA), DVE stt pair-add+accum
# [ZA:4096). GPSIMD cannot run TensorScalarPtr (walrus opcode-on-engine
# check), and ACT+DVE alone already outpace the DMA x-rate.
_ZA = int(os.environ.get("KERNEL_ZA", "1472"))
_WARM1 = int(os.environ.get("KERNEL_WARM1", "6"))
_FILL = int(os.environ.get("KERNEL_FILL", "2"))

WSCALE = float(2 ** 20)   # host scale on Abar/qv/u
W8SHIFT = 0.25            # extra x2^-2 onto w for fp8 range (dr mode)


@functools.lru_cache(maxsize=4)
def _build(p2: str = _P2, za: int = _ZA,
           warm1: int = _WARM1, nfill: int = _FILL):
    _ensure_path()
    import concourse.bass as bass
    import concourse.tile as tile
    from concourse import bacc, mybir

    f32 = mybir.dt.float32
    bf16 = mybir.dt.bfloat16
    f16 = mybir.dt.float16
    f8 = mybir.dt.float8e4
    wdt = f8 if p2 == "dr" else bf16
    ADD = mybir.AluOpType.add
    MUL = mybir.AluOpType.mult
    COPY = mybir.ActivationFunctionType.Copy

    zd = L - za               # DVE stt zone
    zd2 = zd // 2
    # half-tile zones for tile 7 (2048 cols each half)
    ha = za // 2
    hd = H - ha

    nc = bacc.Bacc(
        "TRN2",
        target_bir_lowering=False,
        debug=False,
        enable_asserts=False,
        num_devices=NCORES,
    )

    xT = nc.dram_tensor("xT", [N, L], f8, kind="ExternalInput").ap()
    atr = nc.dram_tensor("atr", [128, PT * N], f8, kind="ExternalInput").ap()
    qv8 = nc.dram_tensor("qv8", [128, PT], f8, kind="ExternalInput").ap()
    u8 = nc.dram_tensor("u8", [128, PT], f32, kind="ExternalInput").ap()
    c0 = nc.dram_tensor("c0", [1, 1], f32, kind="ExternalInput").ap()
    out_d = nc.dram_tensor("out", [1, L], f32, kind="ExternalOutput").ap()

    with tile.TileContext(nc) as tc:
        with (
            tc.tile_pool(name="sb", bufs=1) as sb,
            tc.tile_pool(name="ps", bufs=1, space="PSUM") as ps,
        ):
            xp = [sb.tile([128, 2 * L], f8, tag=f"xp{k}", name=f"xp{k}")
                  for k in range(4)]
            # tile t lives at xp[t//2][:, (t%2)*L : (t%2+1)*L]
            xv = [xp[t // 2][:, (t % 2) * L:(t % 2 + 1) * L] for t in range(8)]
            at0_sb = sb.tile([128, 2 * N], f8, tag="at0")
            at1_sb = sb.tile([128, 6 * N], f8, tag="at1")
            qv_sb = sb.tile([128, PT], f8, tag="qv")
            u_sb = sb.tile([128, PT], f32, tag="u")
            c0_sb = sb.tile([1, 1], f32, tag="c0")
            ones32 = sb.tile([1, 128], f32, tag="ones")
            scrA = sb.tile([128, za], f8, tag="scrA")
            scrD = sb.tile([128, zd2], f16, tag="scrD")
            parts = sb.tile([128, 8, 2], f32, tag="parts")
            parts7 = sb.tile([128, 4], f32, tag="parts7")
            xm7f = sb.tile([128, 1], f32, tag="xm7f")
            xm_all = sb.tile([128, PT], f16, tag="xm")
            w8acc = sb.tile([128, PT], f32, tag="w8acc")
            # DR mode: stationary pair dim must stride %16 and M must be
            # even (s3_lw_dual_fp8_restrictions) -- pad each tile's w
            # column to stride 16 and emit M=2 (second output row junk).
            if p2 == "dr":
                w_sb = sb.tile([128, PT, 16], f8, tag="weff", name="w_sb")
            else:
                w_sb = sb.tile([128, PT], wdt, tag="weff", name="w_sb")
            c_sb = sb.tile([1, 1], f32, tag="csb")
            c_bc = sb.tile([128, 1], f32, tag="cbc")
            out_sb = sb.tile([128, 2 * LCH], f32, tag="osb")

            # PSUM: one tile per bank (pad free dim to a full 2KB bank)
            wp_all = ps.tile([128, 512], f32, tag="wp")     # cols 0:64 used
            warm = ps.tile([1, 512], f32, tag="warm")
            c_ps = ps.tile([1, 512], f32, tag="cps")        # [0:1,0:1] used
            cb_ps = ps.tile([128, 512], f32, tag="cbp")     # col 0 used
            o_ps = [ps.tile([128, LCH], f32, tag=f"o{w}", name=f"o{w}")
                    for w in range(2)]

            # ---- DMA: issues pinned to queue fronts ----
            with tc.high_priority():
                nc.gpsimd.dma_start(at0_sb[:], atr[:, 0:2 * N])
                nc.gpsimd.dma_start(qv_sb[:], qv8[:])
                nc.gpsimd.dma_start(u_sb[:], u8[:])
                nc.gpsimd.dma_start(c0_sb[:], c0[:])
                nc.gpsimd.dma_start(at1_sb[:], atr[:, 2 * N:])
                for k in range(4):
                    nc.sync.dma_start(
                        xp[k][:, 0:L], xT[256 * k:256 * k + 128, :])
                nc.sync.dma_start(xp[3][:, L + H:2 * L], xT[896:, H:L])
                for k in range(3):
                    nc.scalar.dma_start(
                        xp[k][:, L:2 * L], xT[256 * k + 128:256 * (k + 1), :])
                nc.scalar.dma_start(xp[3][:, L:L + H], xT[896:, 0:H])

            nc.gpsimd.memset(ones32[:], 1.0)
            # hoist ACT table load to t~0 via a dummy activation
            nc.scalar.activation(scrA[0:1, 0:8], ones32[0:1, 0:8], COPY,
                                 bias=0.0)
            # pre-zero pass-2 PSUM rows the matvec leaves unwritten (the
            # epilogue reads all 128 partitions; only rows 32j get data)
            for w in range(2):
                nc.vector.memset(o_ps[w][:, :], 0.0)
            if p2 == "dr":
                nc.gpsimd.memset(w_sb[:], 0.0)  # padding cols must be valid

            # ---- per-tile row-sum zones (one op per engine) ----
            def reduce_tile(t):
                x_ = xv[t]
                nc.scalar.activation(
                    scrA[:, 0:za], x_[:, 0:za], COPY, bias=0.0,
                    accum_out=parts[:, t, 0:1])
                nc.vector.scalar_tensor_tensor(
                    scrD[:, 0:zd2], x_[:, za:za + zd2], 1.0,
                    x_[:, za + zd2:L], MUL, ADD,
                    accum_out=parts[:, t, 1:2])

            def reduce_half7(h):  # tile 7 halves -> parts7[:, 2h:2h+2]
                x_ = xv[7][:, h * H:(h + 1) * H]
                nc.scalar.activation(
                    scrA[:, 0:ha], x_[:, 0:ha], COPY, bias=0.0,
                    accum_out=parts7[:, 2 * h + 0:2 * h + 1])
                hd2 = hd // 2
                nc.vector.scalar_tensor_tensor(
                    scrD[:, 0:hd2], x_[:, ha:ha + hd2], 1.0,
                    x_[:, ha + hd2:H], MUL, ADD,
                    accum_out=parts7[:, 2 * h + 1:2 * h + 2])

            def combine(t):  # xm[t] = A + D on GPS, fp16
                nc.gpsimd.tensor_add(
                    xm_all[:, t:t + 1], parts[:, t, 0:1], parts[:, t, 1:2])

            # ---- folds ----
            def fold(pt):
                a_sb, off = (at0_sb, pt) if pt < 2 else (at1_sb, pt - 2)
                for nt in range(PT):
                    nc.tensor.matmul(
                        wp_all[:, pt * 8 + nt:pt * 8 + nt + 1],
                        a_sb[:, off * N + nt * 128:off * N + (nt + 1) * 128],
                        xm_all[:, pt:pt + 1], start=True, stop=True)
                nc.tensor.matmul(
                    c_ps[0:1, 0:1], qv_sb[:, pt:pt + 1], xm_all[:, pt:pt + 1],
                    start=(pt == 0), stop=(pt == PT - 1))
                # PE fillers gated on this xm: keep the HAM clock ramping
                if pt < 6:
                    for i in range(nfill):
                        nc.tensor.matmul(
                            warm[0:1, :], xm_all[:, pt:pt + 1],
                            xv[pt][:, i * LCH:(i + 1) * LCH],
                            start=(i == 0), stop=(i == nfill - 1))

            # warmup burst as soon as x0 lands
            for i in range(warm1):
                nc.tensor.matmul(
                    warm[0:1, :], xv[0][:, 0:1], xv[0][:, 0:LCH],
                    start=(i == 0), stop=(i == warm1 - 1))

            # tiles 0..6 (pair-buffered), tile 7 in halves
            for t in range(7):
                reduce_tile(t)
                if t >= 1:
                    combine(t - 1)
                    fold(t - 1)
            reduce_half7(0)
            combine(6)
            fold(6)
            reduce_half7(1)
            # tile-7 combine on DVE (GPS may lag; DVE is free now)
            nc.vector.tensor_reduce(
                xm7f[:], parts7[:], axis=mybir.AxisListType.X,
                op=mybir.AluOpType.add)
            nc.vector.tensor_copy(xm_all[:, 7:8], xm7f[:])
            fold(7)

            # ---- finalize w / c ----
            nc.vector.tensor_reduce(
                w8acc[:], wp_all[:, 0:64].rearrange("p (a b) -> p b a", a=8),
                axis=mybir.AxisListType.X, op=mybir.AluOpType.add)
            w_dst = w_sb[:, :, 0:1] if p2 == "dr" else w_sb[:]
            nc.vector.scalar_tensor_tensor(
                w_dst, w8acc[:], W8SHIFT if p2 == "dr" else 1.0, u_sb[:],
                MUL, ADD)
            nc.vector.tensor_scalar(
                c_sb[:], c_ps[0:1, 0:1], 1.0 / WSCALE, c0_sb[0:1, 0:1],
                MUL, ADD)
            # broadcast c to all partitions via PE + ACT copy
            nc.tensor.matmul(cb_ps[:, 0:1], ones32[:], c_sb[:],
                             start=True, stop=True)
            nc.scalar.activation(c_bc[:], cb_ps[:, 0:1], COPY, bias=0.0)

            # ---- pass 2 ----
            oscale = (1.0 / (WSCALE * W8SHIFT)) if p2 == "dr" else 1.0 / WSCALE
            for wave in range(2):
                if p2 == "dr":
                    DR = mybir.MatmulPerfMode.DoubleRow
                    for ntp in range(4):
                        xpr = xp[ntp].rearrange("p (two l) -> p two l", two=2)
                        wpr = w_sb[:, 2 * ntp:2 * ntp + 2, 0:2]
                        for j in range(4):
                            lc = wave * 4 + j
                            nc.tensor.matmul(
                                o_ps[wave][32 * j:32 * j + 2, :],
                                wpr, xpr[:, :, lc * LCH:(lc + 1) * LCH],
                                start=(ntp == 0), stop=(ntp == 3),
                                perf_mode=DR, tile_position=(0, 32 * j))
                else:
                    for nt in range(PT):
                        for j in range(4):
                            lc = wave * 4 + j
                            nc.tensor.matmul(
                                o_ps[wave][32 * j:32 * j + 1, :],
                                w_sb[:, nt:nt + 1],
                                xv[nt][:, lc * LCH:(lc + 1) * LCH],
                                start=(nt == 0), stop=(nt == PT - 1),
                                tile_position=(0, 32 * j))
                nc.vector.tensor_scalar(
                    out_sb[:, wave * LCH:(wave + 1) * LCH], o_ps[wave][:, :],
                    oscale, c_bc[:, 0:1], MUL, ADD)
                nc.sync.dma_start(
                    out_d[0:1, wave * 4 * LCH:(wave + 1) * 4 * LCH]
                    .rearrange("p (j k) -> p j k", j=4),
                    out_sb[0:97:32, wave * LCH:(wave + 1) * LCH])

    nc.compile()
    return nc


def _prep_host(inputs, p2=_P2):
    """Fold weights on host (f64 accumulate) and lay out per-core arrays."""
    import ml_dtypes

    Wq = np.asarray(inputs["Wq"], np.float64)
    bq = np.asarray(inputs["bq"], np.float64)
    Wk = np.asarray(inputs["Wk"], np.float64)
    bk = np.asarray(inputs["bk"], np.float64)
    Wfc = np.asarray(inputs["Wfc"], np.float64)
    bfc = np.asarray(inputs["bfc"], np.float64)

    s = np.repeat(Wfc[0], D_K) / np.sqrt(D_K)
    A = (Wk * s[:, None]).T @ Wq / L          # [n, p] ; w_eff = A @ xsum + u
    u = Wk.T @ (s * bq)
    qv = Wq.T @ (s * bk) / L
    c0 = float((s * bk) @ bq + bfc[0])

    f8 = ml_dtypes.float8_e4m3

    at = np.ascontiguousarray(A.T) * WSCALE
    atr = np.ascontiguousarray(
        at.reshape(PT, 128, N).transpose(1, 0, 2).reshape(128, PT * N)
    ).astype(f8)
    qv8 = np.ascontiguousarray(
        (qv * WSCALE).reshape(PT, 128).T).astype(f8)
    uscale = WSCALE * (W8SHIFT if p2 == "dr" else 1.0)
    u8 = np.ascontiguousarray(
        (u * uscale).reshape(PT, 128).T).astype(np.float32)
    c0a = np.full((1, 1), c0, np.float32)

    x = np.asarray(inputs["x"])
    shared = {"atr": atr, "qv8": qv8, "u8": u8, "c0": c0a}
    in_maps = []
    for c in range(NCORES):
        m = dict(shared)
        m["xT"] = np.ascontiguousarray(x[c].T).astype(f8)
        in_maps.append(m)
    return in_maps


LAST_RESULTS = None


def kernel(**inputs) -> np.ndarray:
    global LAST_RESULTS
    _ensure_path()
    from concourse.bass_utils import run_bass_kernel_spmd

    nc = _build()
    in_maps = _prep_host(inputs)
    kw = {}
    if os.environ.get("KERNEL_TRACE"):
        kw["trace"] = True
    res = run_bass_kernel_spmd(nc, in_maps, list(range(NCORES)), **kw)
    LAST_RESULTS = res
    out = np.stack([res.results[c]["out"].reshape(L, 1) for c in range(NCORES)])
    return out.astype(np.float32)


if __name__ == "__main__":
    rng = np.random.default_rng(0)
    demo = {
        "x": rng.standard_normal((B, L, N), np.float32),
        "Wq": rng.standard_normal((N, N), np.float32) * 0.03,
        "bq": rng.standard_normal((N,), np.float32) * 0.03,
        "Wk": rng.standard_normal((N, N), np.float32) * 0.03,
        "bk": rng.standard_normal((N,), np.float32) * 0.03,
        "Wfc": rng.standard_normal((1, 16), np.float32) * 0.25,
        "bfc": rng.standard_normal((1,), np.float32) * 0.25,
    }
    o = kernel(**demo)
    print("out", o.shape, o.dtype, float(np.abs(o).max()))
